# revision 5
# baseline (speedup 1.0000x reference)
"""Trainium2 Bass kernel for nn_EstraNet_1443109012284.

Mathematical reduction: the reference's FAVOR+/trig branch (phi_q, aux_q/k,
fr_q/k, aux_A, A) does not feed the output.  The output is exactly

    out[b,n,d] = sum_{h,c} W_o[h,c,d] * norma[h] * sum_{d'} W_v[d',h,c] * x[b,n,d']
               = (x @ M)[b,n,d],   M[d',d] = sum_{h,c} W_v[d',h,c] norma[h] W_o[h,c,d]

with norma[h] = || sum_d s_p[h] W_p[d,h,:] beta_p[d] ||_2.

M is a tiny [512,512] matrix folded on the host; the device does the single
big GEMM  y[32768,512] = x[32768,512] @ M[512,512]  data-parallel over rows:
each of the 8 cores handles 4096 rows.

Device design (per core), v2 (streaming): compute yT[d, n] = sum_k M[k,d] xT[k,n]
- fp16 everywhere; M pre-scaled by an exact power of two (undone on host).
- Loop h(4 n-quarters) -> k(4 contraction chunks) -> d(4 out rows) -> j(2
  512-col halves).  PSUM: 8 banks = (d,j) per quarter, accumulated across k.
  First MMs need only m[k0] (128KB) + x[h0,k0] (256KB) -> PE starts ~2us.
- PSUM->SBUF copies on DVE (vector): (512+58)/0.96 ~ 0.6us per bank, single
  PSUM reader (two concurrent readers throttle the PE ~2.3x).
- Input DMAs get dedicated HWDGE rings (sync: h0,h1; scalar: m,h2,h3) so x
  never queues behind output stores.  Outputs go mostly to gpsimd SWDGE with
  late tiles on sync/scalar after their input queues drain.
- Final tile's two 512-col halves store separately on the two HWDGE rings so
  the kernel tail is copy(0.6us) + small DMA, not a serialized 1MB flush.
- PE warmed with dummy matmuls gated only on a memset tile to burn the HAM
  cold-clock ramp while the first input DMAs fly.
"""

import os as _os
import sys

sys.path.insert(0, "/opt/trn_rl_repo")

import numpy as np

import concourse.bass as bass
import concourse.tile as tile
from concourse import bacc, mybir
from concourse.bass_utils import run_bass_kernel_spmd

N_CORES = 8
ROWS = 32768           # B*N = 8*4096
RPC = ROWS // N_CORES  # rows per core = 4096
D = 512
KC = 4                 # contraction chunks of 128
DT = D // 128          # output row-blocks = 4
HB = 4                 # n-quarters per stripe
HW = RPC // HB         # 1024 columns per quarter
JH = HW // 512         # moving chunks of 512 per quarter = 2

COMPUTE_DTYPE = _os.environ.get("KERNEL_DTYPE", "fp16")
N_WARM = int(_os.environ.get("KERNEL_NWARM", "5"))

_DT = {
    "fp32": mybir.dt.float32,
    "f32r": mybir.dt.float32r,
    "bf16": mybir.dt.bfloat16,
    "fp16": mybir.dt.float16,
}


def _np_dtype(token):
    if token == "bf16":
        import ml_dtypes

        return ml_dtypes.bfloat16
    if token == "fp16":
        return np.float16
    return np.float32


def _build(token):
    dt_in = _DT[token]
    dt_out = mybir.dt.float16 if token == "fp16" else mybir.dt.float32
    nc = bacc.Bacc("TRN2", target_bir_lowering=False)
    # x pre-transposed, [h, k, 128, 1024]: chunk (h,k) contiguous 256KB
    xt = nc.dram_tensor("xt", [HB, KC, 128, HW], dt_in, kind="ExternalInput")
    mm = nc.dram_tensor("mm", [128, KC, D], dt_in, kind="ExternalInput")
    yt = nc.dram_tensor("yt", [D, RPC], dt_out, kind="ExternalOutput")

    with tile.TileContext(nc) as tc:
        with (
            tc.tile_pool(name="xp", bufs=1) as xp,
            tc.tile_pool(name="mp", bufs=1) as mp,
            tc.tile_pool(name="op", bufs=4) as op,
            tc.tile_pool(name="pp", bufs=8, space="PSUM") as pp,
        ):
            # PE warmup: matmuls gated only on a local memset burn the HAM
            # cold-clock ramp while the first input DMAs are in flight.
            wz = mp.tile([128, 512], mybir.dt.bfloat16, name="wz")
            nc.gpsimd.memset(wz[:], 1.0)
            warm = pp.tile([128, 512], mybir.dt.float32, tag="ps", name="warm")
            for w in range(N_WARM):
                nc.tensor.matmul(
                    warm[:], wz[:, 0:128], wz[:], start=True, stop=True
                )

            # m[k0] first on sync so the first MM gates on a single ring;
            # the other three k slices ride scalar ahead of h2/h3.
            m_sb = mp.tile([128, KC, D], dt_in, name="m_sb")
            nc.sync.dma_start(out=m_sb[:, 0, :], in_=mm[:, 0, :])
            nc.scalar.dma_start(out=m_sb[:, 1:KC, :], in_=mm[:, 1:KC, :])

            # x inputs: sync carries h0,h1; scalar carries h2,h3 (after m).
            x_sb = {}
            for h in range(HB):
                for k in range(KC):
                    t = xp.tile([128, HW], dt_in, tag=f"x{h}{k}", name=f"x{h}{k}")
                    eng = nc.sync if h < 2 else nc.scalar
                    eng.dma_start(out=t[:], in_=xt[h, k])
                    x_sb[(h, k)] = t

            # output engine per (h,d) tile: gpsimd early (HWDGE rings still
            # pulling inputs), rotate later so no ring serializes
            G, S, C = nc.gpsimd, nc.sync, nc.scalar
            OENG = [
                G, G, G, G,
                S, C, G, S,
                C, S, G, C,
                S, C, G, None,  # last tile split j-wise below
            ]

            def copy_eng(j):
                # one PSUM reader per bank: ACT drains j0 banks, DVE j1 banks
                return nc.scalar.copy if j == 0 else nc.vector.tensor_copy

            for h in range(HB):
                pss = [
                    pp.tile([128, 512], mybir.dt.float32, tag="ps",
                            name=f"ps_{h}_{dj // JH}_{dj % JH}")
                    for dj in range(DT * JH)
                ]
                # h0: k-outer so MMs start after just m[k0]+x[h0,k0];
                # h1+: d-outer so each (d,j) tile closes early and its
                # copy/store spreads instead of bunching at quarter end.
                if h == 0:
                    for k in range(KC):
                        for d in range(DT):
                            for j in range(JH):
                                nc.tensor.matmul(
                                    pss[d * JH + j][:],
                                    m_sb[:, k, d * 128 : (d + 1) * 128],
                                    x_sb[(h, k)][:, j * 512 : (j + 1) * 512],
                                    start=(k == 0),
                                    stop=(k == KC - 1),
                                )
                for d in range(DT):
                    if h > 0:
                        for k in range(KC):
                            for j in range(JH):
                                nc.tensor.matmul(
                                    pss[d * JH + j][:],
                                    m_sb[:, k, d * 128 : (d + 1) * 128],
                                    x_sb[(h, k)][:, j * 512 : (j + 1) * 512],
                                    start=(k == 0),
                                    stop=(k == KC - 1),
                                )
                    ot = op.tile([128, HW], dt_out, name=f"ot{h}{d}", tag="ot")
                    last = h == HB - 1 and d == DT - 1
                    if last:
                        # final tile: quarter-granularity copies and stores,
                        # split across both HWDGE rings to shorten the tail
                        for j in range(JH):
                            cp = copy_eng(j)
                            seng = nc.sync if j == 0 else nc.scalar
                            for q in range(2):
                                c0 = j * 512 + q * 256
                                cp(ot[:, c0 : c0 + 256],
                                   pss[d * JH + j][:, q * 256 : (q + 1) * 256])
                                seng.dma_start(
                                    out=yt[d * 128 : (d + 1) * 128,
                                           h * HW + c0 : h * HW + c0 + 256],
                                    in_=ot[:, c0 : c0 + 256],
                                )
                    else:
                        for j in range(JH):
                            copy_eng(j)(
                                ot[:, j * 512 : (j + 1) * 512],
                                pss[d * JH + j][:],
                            )
                        OENG[h * DT + d].dma_start(
                            out=yt[d * 128 : (d + 1) * 128, h * HW : (h + 1) * HW],
                            in_=ot[:],
                        )
    nc.compile()
    return nc


def _fold_m(W_v, s_p, W_p, beta_p, W_o):
    """Host-side constant folding of the tiny parameter tensors into M."""
    W_v = np.asarray(W_v, dtype=np.float64)
    s_p = np.asarray(s_p, dtype=np.float64)
    W_p = np.asarray(W_p, dtype=np.float64)
    beta_p = np.asarray(beta_p, dtype=np.float64)
    W_o = np.asarray(W_o, dtype=np.float64)
    phi = np.einsum("h,dhc,d->hc", s_p, W_p, beta_p)
    norma = np.linalg.norm(phi, axis=1)  # [h]
    M = np.einsum("dhc,h,hce->de", W_v, norma, W_o)  # [512, 512]
    return M.astype(np.float32)


_prog_cache = {}
_last_in_maps = None  # kept for test.py profiling reuse
_last_result = None


def _run(in_maps, token, **kwargs):
    if token not in _prog_cache:
        _prog_cache[token] = _build(token)
    return run_bass_kernel_spmd(_prog_cache[token], in_maps, list(range(N_CORES)), **kwargs)


def kernel(x, W_v, s_p, c_p, W_p, W_A, W_o, beta_p, beta_i_p, **_unused):
    global _last_in_maps, _last_result
    token = COMPUTE_DTYPE
    np_dt = _np_dtype(token)

    x = np.asarray(x, dtype=np.float32)
    M = _fold_m(W_v, s_p, W_p, beta_p, W_o)

    # fp16 path: scale M by an exact power of two so M entries and y values
    # sit in fp16 normal range; undo on the host after the run
    out_unscale = 1.0
    if token == "fp16":
        amax = float(np.abs(M).max())
        if amax > 0:
            e = int(np.floor(-np.log2(amax)))
            M = M * np.float32(2.0**e)
            out_unscale = 2.0**-e

    B, N, Dd = x.shape
    assert B * N == ROWS and Dd == D, (x.shape,)

    mmc = np.ascontiguousarray(M.reshape(KC, 128, D).transpose(1, 0, 2)).astype(np_dt)
    xf = x.reshape(ROWS, D)

    in_maps = []
    for c in range(N_CORES):
        sh = xf[c * RPC : (c + 1) * RPC]               # [4096, 512]
        xT = sh.T.astype(np_dt)                        # [512, 4096]
        # [KC, 128, HB, HW] -> [HB, KC, 128, HW], chunk (h,k) contiguous
        xs = np.ascontiguousarray(
            xT.reshape(KC, 128, HB, HW).transpose(2, 0, 1, 3)
        )
        in_maps.append({"xt": xs, "mm": mmc})

    _last_in_maps = in_maps
    res = _run(in_maps, token)
    _last_result = res
    out = np.empty((ROWS, D), dtype=np.float32)
    for c in range(N_CORES):
        yc = res.results[c]["yt"].astype(np.float32)
        if out_unscale != 1.0:
            yc *= np.float32(out_unscale)
        out[c * RPC : (c + 1) * RPC] = yc.T
    return out.reshape(B, N, D)


if __name__ == "__main__":
    # smoke test with random data
    rng = np.random.default_rng(0)
    x = rng.standard_normal((8, 4096, 512)).astype(np.float32)
    W_v = rng.standard_normal((512, 8, 64)).astype(np.float32) * 0.01
    s_p = np.ones((8,), np.float32)
    c_p = np.ones((8,), np.float32)
    W_p = rng.standard_normal((512, 8, 64)).astype(np.float32) * 0.01
    W_A = rng.standard_normal((256, 64)).astype(np.float32)
    W_o = rng.standard_normal((8, 64, 512)).astype(np.float32) * 0.01
    beta_p = rng.standard_normal((512,)).astype(np.float32) * 1e-5
    beta_i_p = rng.standard_normal((4096, 512)).astype(np.float32) * 1e-5
    out = kernel(x, W_v=W_v, s_p=s_p, c_p=c_p, W_p=W_p, W_A=W_A, W_o=W_o,
                 beta_p=beta_p, beta_i_p=beta_i_p)
    M = _fold_m(W_v, s_p, W_p, beta_p, W_o)
    exp = (x.reshape(-1, 512).astype(np.float64) @ M.astype(np.float64)).reshape(8, 4096, 512)
    err = np.abs(out - exp).max() / (np.abs(exp).max() + 1e-30)
    print("smoke rel err:", err)
